# revision 87
# baseline (speedup 1.0000x reference)
"""MoE block (KlearSparseMoeBlock) on 8 trn2 NeuronCores.

Strategy (expert-parallel, per sharding hint):
  - Host computes the sigmoid router + top-k and realizes the all-to-all
    dispatch at the sharding step: experts are assigned to (core, slot)
    serpentine-by-descending-count so per-slot capacities are tight
    (caps ~ [296, 272, 256, 256] vs uniform 320), and each core gets
    pre-gathered transposed activations xcT for its 4 experts.
  - Device (per core): dense SwiGLU per expert in bf16 with fp32 PSUM.
    The down-projection is computed TRANSPOSED (yT[d, slot]) so the
    per-slot combine weights fold into the host scatter-add — no
    per-slot vector work on device.
  - Shared expert is sharded 2-way over hidden (halves) x 4-way over
    tokens (quarters): core c computes hidden-half c//4 for token
    quarter c%4. The coef[:,1] scale also folds into the host combine.
  - Scheduling: shared up/gate matmuls run first (high PE-work-per-byte,
    covers the DMA ramp; a few zero matmuls pre-warm the PE clock), then
    experts with shared down-proj tiles slotted mid-mm2. The last
    expert's weights ride in a pool reusing the freed up/gate SBUF, and
    its down tile runs before its mm1 to cover the weight arrival.
    DMA issue order == consumption order; first chunks are small so the
    first matmul starts ~3 us in. PSUM: per-hh sibling pools release
    banks to expert psums as each swiglu drains (mm1 x2, mm2 x4, down
    x2 rotation fills all 8 banks).
  - Host combine: out = cf1 * (ysh_lo + ysh_hi) per token quarter, plus
    scatter-add of w_slot * yT columns per expert.

TimelineSim (per-core cost model): ~136.1 us vs ~224.5 us for the staged
baseline (1.65x); PE ~93% busy, DMA ~90% busy (compute roofline ~127 us,
memory ~124 us — ridge regime, both near saturation). Matmul free dims use
true slot counts (frees) while the DMA layout keeps 256-padded caps (the
512B-run rule); the host ignores the padded tail columns. Remaining
overhead: ~3.6 us DMA-gated startup (SP preamble + DMA init + first chunk
transfer + semaphore), ~3.9 us output-drain tail (copy + descriptor +
transfer + engine-drain barriers).
"""

import functools

import numpy as np
import ml_dtypes

BF16 = ml_dtypes.bfloat16

# problem shapes (hardcoded per contract)
D = 2048      # model dim
H = 512       # expert hidden
E = 32        # experts
HS = 1024     # shared hidden
S = 2048      # tokens
NCORES = 8
EPC = E // NCORES          # experts per core = 4
HH = HS // 2               # shared hidden half = 512
TQ = S // 4                # shared token quarter = 512
P = 128
DC = D // P                # 16 d-chunks
HC = H // P                # 4 h-chunks per expert hidden


@functools.lru_cache(maxsize=4)
def _build_program(caps, frees=None):
    if frees is None:
        frees = caps
    import concourse.tile as tile
    from concourse import bacc, mybir

    f32 = mybir.dt.float32
    bf16 = mybir.dt.bfloat16

    slots = sum(caps)
    offs = [sum(caps[:i]) for i in range(EPC)]

    # Bacc (not raw Bass): its compile pipeline splits multi-sem waits into
    # event semaphores — TRN2 allows at most one wait per instruction.
    nc = bacc.Bacc(None)

    # ---- per-core inputs ----
    xcT_d = nc.declare_dram_parameter("xcT", [D, slots], bf16, isOutput=False)
    wg_d = nc.declare_dram_parameter("wgc", [EPC, D, H], bf16, isOutput=False)
    wu_d = nc.declare_dram_parameter("wuc", [EPC, D, H], bf16, isOutput=False)
    wd_d = nc.declare_dram_parameter("wdc", [EPC, H, D], bf16, isOutput=False)
    # sg / xTs packed along one axis (the startup feed is issue/descriptor
    # bound, not byte-bound); su separate so the first chunk — which gates
    # the first g-matmul — is a third smaller
    shp_d = nc.declare_dram_parameter("shp", [D, 2, HH], bf16, isOutput=False)
    su_d = nc.declare_dram_parameter("suc", [D, HH], bf16, isOutput=False)
    sd_d = nc.declare_dram_parameter("sdc", [HH, D], bf16, isOutput=False)

    # ---- per-core outputs ----
    yT_d = nc.declare_dram_parameter("yT", [D, slots], bf16, isOutput=True)
    ysh_d = nc.declare_dram_parameter("ysh", [TQ, D], bf16, isOutput=True)

    silu = mybir.ActivationFunctionType.Silu
    copyf = mybir.ActivationFunctionType.Copy
    mult = mybir.AluOpType.mult

    with tile.TileContext(nc) as tc:
        with (
            # expert streaming pools (outer: coexist with ug scope)
            tc.tile_pool(name="wgp", bufs=2) as wgp,
            tc.tile_pool(name="wup", bufs=2) as wup,
            tc.tile_pool(name="wdp", bufs=3) as wdp,
            tc.tile_pool(name="xcp", bufs=2) as xcp,
            tc.tile_pool(name="acp", bufs=2) as acp,
            tc.tile_pool(name="sgt", bufs=2) as sgtp,
            tc.tile_pool(name="ystp", bufs=4) as ystp,
            tc.tile_pool(name="sdp", bufs=1) as sdp,
            tc.tile_pool(name="astp", bufs=1) as astp,
            tc.tile_pool(name="yshp", bufs=3) as yshp,
        ):
            # ---------- emit ALL input DMAs in consumption order (SP) ----------
            # shared up/gate inputs first (they feed the PE during the ramp)
            sg_sb = None
            with tc.tile_pool(name="ugsb", bufs=1) as ugsb:
                ug_sb = ugsb.tile([P, DC, 2, HH], bf16, tag="ugp")
                su_sb = ugsb.tile([P, DC, HH], bf16, tag="sup")
                # smaller leading chunks so the first matmul starts ASAP
                edges = [0, 1, 2, 4, 8, 12, 16]
                for kg in range(len(edges) - 1):
                    k0, k1 = edges[kg], edges[kg + 1]
                    rs = slice(k0 * P, k1 * P)
                    nc.sync.dma_start(
                        out=ug_sb[:, k0:k1, :, :],
                        in_=shp_d[rs, :, :].rearrange("(c p) t h -> p c t h", p=P),
                    )
                    nc.sync.dma_start(
                        out=su_sb[:, k0:k1, :],
                        in_=su_d[rs, :].rearrange("(c p) h -> p c h", p=P),
                    )

                # expert inputs + sd, interleaved in consumption order
                wg_sb = []
                wu_sb = []
                wd_sb = []
                xc_sb = []
                sd_sb = sdp.tile([P, 4, D], bf16, tag="sd")
                for e in range(EPC):
                    cap = caps[e]
                    xc = xcp.tile([P, DC, cap], bf16, tag="xc")
                    nchunk = 4 if e in (0, EPC - 1) else 1
                    cw = DC // nchunk
                    for kg in range(nchunk):
                        rs = slice(kg * cw * P, (kg + 1) * cw * P)
                        cs = slice(kg * cw, (kg + 1) * cw)
                        nc.sync.dma_start(
                            out=xc[:, cs, :],
                            in_=xcT_d[rs, offs[e] : offs[e] + cap].rearrange(
                                "(c p) t -> p c t", p=P
                            ),
                        )
                    if e < EPC - 1:
                        # last expert's wg/wu tiles live in the pool that
                        # reuses the ug SBUF space (no WAR on earlier experts)
                        wgt = wgp.tile([P, DC, H], bf16, tag="wg")
                        wut = wup.tile([P, DC, H], bf16, tag="wu")
                        for kg in range(nchunk):
                            rs = slice(kg * cw * P, (kg + 1) * cw * P)
                            cs = slice(kg * cw, (kg + 1) * cw)
                            nc.sync.dma_start(
                                out=wgt[:, cs, :],
                                in_=wg_d[e, rs, :].rearrange("(c p) h -> p c h", p=P),
                            )
                        for kg in range(nchunk):
                            rs = slice(kg * cw * P, (kg + 1) * cw * P)
                            cs = slice(kg * cw, (kg + 1) * cw)
                            nc.sync.dma_start(
                                out=wut[:, cs, :],
                                in_=wu_d[e, rs, :].rearrange("(c p) h -> p c h", p=P),
                            )
                    else:
                        wgt = None
                        wut = None
                    wdt = (
                        wdp.tile([P, 2, D], bf16, tag="wd", name="wdA"),
                        wdp.tile([P, 2, D], bf16, tag="wd", name="wdB"),
                    )
                    if e < EPC - 1:
                        for half in range(2):
                            nc.sync.dma_start(
                                out=wdt[half][:],
                                in_=wd_d[
                                    e, half * 2 * P : (half + 1) * 2 * P, :
                                ].rearrange("(c p) d -> p c d", p=P),
                            )
                    if e == 0:
                        # sd arrives after expert-0 inputs, before expert 1
                        nc.sync.dma_start(
                            out=sd_sb[:],
                            in_=sd_d[:].rearrange("(c p) d -> p c d", p=P),
                        )
                    wg_sb.append(wgt)
                    wu_sb.append(wut)
                    wd_sb.append(wdt)
                    xc_sb.append(xc)

                # ---------- shared up/gate (fills PE during DMA ramp) ----------
                asT = [
                    astp.tile([P, TQ], bf16, tag=f"asT{hh}", name=f"asT{hh}")
                    for hh in range(4)
                ]
                # warm up the PE clock (p-state ramp) on zeros while the
                # first input DMAs are still in flight
                warm = astp.tile([P, P], bf16, tag="warm")
                nc.gpsimd.memset(warm[:], 0)
                with (
                    tc.tile_pool(name="ugps0", bufs=1, space="PSUM") as ugps0,
                    tc.tile_pool(name="ugps1", bufs=1, space="PSUM") as ugps1,
                    tc.tile_pool(name="ugps2", bufs=1, space="PSUM") as ugps2,
                    tc.tile_pool(name="ugps3", bufs=1, space="PSUM") as ugps3,
                ):
                    ugps = [ugps0, ugps1, ugps2, ugps3]
                    # one sibling pool per hh so each releases independently:
                    # expert psums only wait on the matching ug drain
                    pgs = [
                        ugps[hh].tile([P, TQ], f32, tag=f"spg{hh}", name=f"spg{hh}")
                        for hh in range(4)
                    ]
                    pus = [
                        ugps[hh].tile([P, TQ], f32, tag=f"spu{hh}", name=f"spu{hh}")
                        for hh in range(4)
                    ]
                    for _ in range(8):
                        nc.tensor.matmul(
                            out=pgs[0][:, :P],
                            lhsT=warm[:],
                            rhs=warm[:],
                            start=True,
                            stop=True,
                        )
                    for k in range(DC):
                        # last k-step: pair g/u per hh so drain hh0 finishes
                        # (and frees its banks for expert psums) earliest
                        order = (
                            [(w, hh) for hh in range(4) for w in (0, 1)]
                            if k == DC - 1
                            else [(w, hh) for w in (0, 1) for hh in range(4)]
                        )
                        for w, hh in order:
                            lhsT = (
                                ug_sb[:, k, 0, hh * P : (hh + 1) * P]
                                if w == 0
                                else su_sb[:, k, hh * P : (hh + 1) * P]
                            )
                            nc.tensor.matmul(
                                out=(pgs if w == 0 else pus)[hh][:],
                                lhsT=lhsT,
                                rhs=ug_sb[:, k, 1, :],
                                start=(k == 0),
                                stop=(k == DC - 1),
                            )
                    for hh in range(4):
                        sgt = sgtp.tile([P, TQ], bf16, tag="sgt")
                        nc.scalar.activation(out=sgt[:], in_=pgs[hh][:], func=silu)
                        nc.vector.scalar_tensor_tensor(
                            out=asT[hh][:],
                            in0=pus[hh][:],
                            scalar=1.0,
                            in1=sgt[:],
                            op0=mult,
                            op1=mult,
                        )

            # ---------- experts with shared-down tiles interleaved ----------
            with (
                tc.tile_pool(name="e3wp", bufs=1) as e3wp,
                tc.tile_pool(name="mm1", bufs=1, space="PSUM") as mm1,
                tc.tile_pool(name="mm2", bufs=4, space="PSUM") as mm2,
                tc.tile_pool(name="dwn", bufs=2, space="PSUM") as dwn,
            ):
                # last expert's weights go into the SBUF freed by the ug pool;
                # fresh tiles => the DMAs issue as soon as SP reaches them
                wg_sb[EPC - 1] = e3wp.tile([P, DC, H], bf16, tag="wg3", name="wg3")
                wu_sb[EPC - 1] = e3wp.tile([P, DC, H], bf16, tag="wu3", name="wu3")
                e3 = EPC - 1
                for kg in range(4):
                    rs = slice(kg * 4 * P, (kg + 1) * 4 * P)
                    cs = slice(kg * 4, (kg + 1) * 4)
                    nc.sync.dma_start(
                        out=wg_sb[e3][:, cs, :],
                        in_=wg_d[e3, rs, :].rearrange("(c p) h -> p c h", p=P),
                    )
                for kg in range(4):
                    rs = slice(kg * 4 * P, (kg + 1) * 4 * P)
                    cs = slice(kg * 4, (kg + 1) * 4)
                    nc.sync.dma_start(
                        out=wu_sb[e3][:, cs, :],
                        in_=wu_d[e3, rs, :].rearrange("(c p) h -> p c h", p=P),
                    )
                for half in range(2):
                    nc.sync.dma_start(
                        out=wd_sb[e3][half][:],
                        in_=wd_d[e3, half * 2 * P : (half + 1) * 2 * P, :].rearrange(
                            "(c p) d -> p c d", p=P
                        ),
                    )
                for e in range(EPC):
                    cap = caps[e]
                    fr = frees[e]
                    acT = acp.tile([P, HC, fr], bf16, tag="acT")

                    def down_tile(tt):
                        ysh_sb = yshp.tile([P, D], bf16, tag="ysh", name="ysh")
                        for dd in range(4):
                            ps = dwn.tile([P, 512], f32, tag="ps", name="ps")
                            for hh in range(4):
                                nc.tensor.matmul(
                                    out=ps[:],
                                    lhsT=asT[hh][:, tt * P : (tt + 1) * P],
                                    rhs=sd_sb[:, hh, dd * 512 : (dd + 1) * 512],
                                    start=(hh == 0),
                                    stop=(hh == 3),
                                )
                            nc.vector.tensor_copy(
                                ysh_sb[:, dd * 512 : (dd + 1) * 512], ps[:]
                            )
                        nc.scalar.dma_start(
                            out=ysh_d[tt * P : (tt + 1) * P, :], in_=ysh_sb[:]
                        )

                    if e == EPC - 1:
                        # last expert: its weights arrive last (DMA queue is
                        # saturated in the back half); fill the wait with the
                        # shared-down tile, which needs only resident data
                        down_tile(e)
                    # mm1: up/gate for this expert
                    for hc in range(HC):
                        pg = mm1.tile([P, fr], f32, tag="pg")
                        pu = mm1.tile([P, fr], f32, tag="pu")
                        for k in range(DC):
                            nc.tensor.matmul(
                                out=pg[:],
                                lhsT=wg_sb[e][:, k, hc * P : (hc + 1) * P],
                                rhs=xc_sb[e][:, k, :fr],
                                start=(k == 0),
                                stop=(k == DC - 1),
                            )
                        for k in range(DC):
                            nc.tensor.matmul(
                                out=pu[:],
                                lhsT=wu_sb[e][:, k, hc * P : (hc + 1) * P],
                                rhs=xc_sb[e][:, k, :fr],
                                start=(k == 0),
                                stop=(k == DC - 1),
                            )
                        sgt = sgtp.tile([P, fr], bf16, tag="sgt")
                        nc.scalar.activation(out=sgt[:], in_=pg[:], func=silu)
                        nc.vector.scalar_tensor_tensor(
                            out=acT[:, hc, :],
                            in0=pu[:],
                            scalar=1.0,
                            in1=sgt[:],
                            op0=mult,
                            op1=mult,
                        )
                    # mm2 (transposed): yT[d, slot] per d-chunk group; the
                    # shared-down token tile is slotted mid-mm2 so the
                    # program tail ends on the short yT chain, not ysh.
                    # Last expert ends on two half-groups so the final
                    # copy->DMA->drain chain is as short as possible.
                    if e == EPC - 1:
                        groups = [(0, 4), (4, 4), (8, 4), (12, 3), (15, 1)]
                    else:
                        groups = [(0, 4), (4, 4), (8, 4), (12, 4)]
                    for gi, (kd0, gn) in enumerate(groups):
                        if gi == 2 and e < EPC - 1:
                            down_tile(e)
                        yst = ystp.tile([P, gn, cap], bf16, tag="yst", name="yst")
                        for j in range(gn):
                            kd = kd0 + j
                            py = mm2.tile([P, fr], f32, tag="py")
                            for hs in range(HC):
                                nc.tensor.matmul(
                                    out=py[:],
                                    lhsT=wd_sb[e][hs // 2][
                                        :, hs % 2, kd * P : (kd + 1) * P
                                    ],
                                    rhs=acT[:, hs, :],
                                    start=(hs == 0),
                                    stop=(hs == HC - 1),
                                )
                            # first group: lead with Act twice (DVE is still
                            # draining the last swiglu at mm2 start); final
                            # half-group: DVE, so Act issues the DMA at once
                            if gi == 0:
                                on_act = j < 2
                            elif e == EPC - 1 and gi == len(groups) - 1:
                                on_act = False
                            else:
                                on_act = j % 2 == 0
                            # copies cover only [:fr]; the DMA ships the full
                            # cap-padded tile and the host ignores the tail
                            if on_act:
                                nc.scalar.activation(
                                    out=yst[:, j, :fr], in_=py[:], func=copyf
                                )
                            else:
                                nc.vector.tensor_copy(yst[:, j, :fr], py[:])
                        nc.scalar.dma_start(
                            out=yT_d[
                                kd0 * P : (kd0 + gn) * P,
                                offs[e] : offs[e] + cap,
                            ].rearrange("(c p) t -> p c t", p=P),
                            in_=yst[:],
                        )

    if not nc.is_finalized():
        nc.finalize()  # Bacc: runs compile() (reg alloc, event-sem wait split)
    return nc


def _round_cap(n):
    return max(256, (n + 3) // 4 * 4)


def kernel(x, gate_w, expert_bias, wg, wu, wd, sg, su, sd, coef_w, coef_b, top_k):
    from concourse.bass_utils import run_bass_kernel_spmd

    x2 = np.ascontiguousarray(np.asarray(x, dtype=np.float32).reshape(S, D))
    gate_w = np.asarray(gate_w, dtype=np.float32)
    expert_bias = np.asarray(expert_bias, dtype=np.float32)
    coef_w = np.asarray(coef_w, dtype=np.float32)
    coef_b = np.asarray(coef_b, dtype=np.float32)
    top_k = int(top_k)

    # ---- router (host; 0.3% of total FLOPs) ----
    logits = x2 @ gate_w.T
    routing = 1.0 / (1.0 + np.exp(-logits))
    biased = routing + expert_bias[None, :]
    inds = np.argpartition(-biased, top_k - 1, axis=-1)[:, :top_k]  # [S,K]
    scores = np.take_along_axis(routing, inds, axis=-1)
    wnorm = scores / (scores.sum(-1, keepdims=True) + 1e-20)

    cl = x2 @ coef_w.T + coef_b[None, :]
    cl -= cl.max(-1, keepdims=True)
    ce = np.exp(cl)
    coef = ce / ce.sum(-1, keepdims=True)  # [S,2]

    # ---- expert -> (core, slot) serpentine assignment ----
    counts = np.array([(inds == e).sum() for e in range(E)], np.int64)
    order = np.argsort(-counts, kind="stable")
    slot_max = [
        int(counts[order[s * NCORES : (s + 1) * NCORES]].max()) for s in range(EPC)
    ]
    caps = tuple(_round_cap(n) for n in slot_max)
    # matmul free dims: true counts (rounded to 4) — the 256 floor on caps
    # is only for DMA run lengths; compute can stop at the real slot count
    frees = tuple(min(c, (n + 3) // 4 * 4) for c, n in zip(caps, slot_max))
    assert all(c <= 512 for c in caps), f"caps {caps} exceed psum bank"
    slots = sum(caps)
    offs = [sum(caps[:i]) for i in range(EPC)]

    xbf = x2.astype(BF16)
    xT = np.ascontiguousarray(xbf.T)  # [D, S]

    tok = {}   # expert -> (rows, combine weights)
    for e in range(E):
        rows, cols = np.nonzero(inds == e)
        tok[e] = (rows, wnorm[rows, cols] * coef[rows, 0])

    in_maps = []
    for c in range(NCORES):
        experts_c = [int(order[s * NCORES + c]) for s in range(EPC)]
        xcT = np.zeros((D, slots), BF16)
        for s, e in enumerate(experts_c):
            rows, _ = tok[e]
            xcT[:, offs[s] : offs[s] + len(rows)] = xT[:, rows]
        hh, tq = divmod(c, 4)
        shp = np.empty((D, 2, HH), BF16)
        shp[:, 0, :] = sg[:, hh * HH : (hh + 1) * HH]
        shp[:, 1, :] = xT[:, tq * TQ : (tq + 1) * TQ]
        in_maps.append(
            {
                "xcT": xcT,
                "wgc": wg[experts_c].astype(BF16),
                "wuc": wu[experts_c].astype(BF16),
                "wdc": wd[experts_c].astype(BF16),
                "shp": shp,
                "suc": np.ascontiguousarray(su[:, hh * HH : (hh + 1) * HH]).astype(BF16),
                "sdc": np.ascontiguousarray(sd[hh * HH : (hh + 1) * HH, :]).astype(BF16),
            }
        )

    nc = _build_program(caps, frees)
    global LAST_CAPS, LAST_FREES
    LAST_CAPS = caps
    LAST_FREES = frees
    import os

    trace = bool(os.environ.get("KERNEL_TRACE"))
    res = run_bass_kernel_spmd(nc, in_maps, list(range(NCORES)), trace=trace)
    global LAST_EXEC_NS
    LAST_EXEC_NS = res.exec_time_ns
    results = res.results

    # ---- combine ----
    out = np.zeros((S, D), np.float32)
    cf1 = coef[:, 1]
    for tq in range(4):
        sh = results[tq]["ysh"].astype(np.float32) + results[4 + tq]["ysh"].astype(
            np.float32
        )
        out[tq * TQ : (tq + 1) * TQ] = sh * cf1[tq * TQ : (tq + 1) * TQ, None]
    for c in range(NCORES):
        yT = results[c]["yT"]
        for s in range(EPC):
            e = int(order[s * NCORES + c])
            rows, w = tok[e]
            seg = yT[:, offs[s] : offs[s] + len(rows)].T.astype(np.float32)
            out[rows] += w[:, None] * seg
    return out.reshape(1, S, D).astype(np.float32)
